# revision 34
# baseline (speedup 1.0000x reference)
"""FullAttention (non-standard multiplicative causal mask) on 8 TRN2 cores.

Reference (per batch b, head h):
    S = Q @ K^T                      [L, L]
    S = S * tril(ones)               (multiplicative mask: zeros above diag)
    A = softmax(S / sqrt(E))         (masked slots contribute exp(0)=1)
    O = A @ V

Key observation: for s > l, P[l,s] = exp(0) = 1, so
    num[l]   = sum_{s<=l} exp(z_ls) v_s  +  sum_{s>l} v_s
    denom[l] = sum_{s<=l} exp(z_ls)      +  (L-1-l)

Sharding: B*H = 32 (b,h) pairs -> 4 per core (2 "packs" of 2 heads).
Host pre-transposes Q,K to [e, l] layout (bf16); 2 heads are stacked on
the 128 SBUF partitions so QK^T matmuls (contraction E=64) row-pack the
PE array.  Q/K/V are loaded in chunk-sized slices so the first QK
matmul starts as soon as the first 512 columns land.

Per (b,h), chunk-outer flash-style loop over l-chunks of 512:
  - QK: S^T pieces [s_tile=128, <=512] via matmul(lhsT=kT, rhs=qT)
  - exp on ScalarE (PSUM->SBUF, bf16 out), causal wedge fixup via
    gpsimd affine_select (exp(0)=1 for s>l inside the diagonal tile)
  - PV: out^T [66, 512] += V1[s_tile].T @ P^T piece  (V1 = [V | 1 | 0]
    adds the softmax denominator as column 64)
  - the [66, 512] accumulator is copied to SBUF bf16 and DMA'd out
    untransposed; the host adds the precomputed suffix sums
    (s_tiles > l's tile), divides by the denominator and transposes.

The PE is power-throttled to ~50% sustained utilization on TRN2, so
total streamed matmul columns (2x causal area) set the span floor; the
warmup ramps the p-state during the input-DMA window using small-N
matmuls (ramp needs busy-time, not columns; columns burn throttle
credit).
"""

import numpy as np

import concourse.bass as bass
import concourse.mybir as mybir
import concourse.tile as tile
from concourse import bacc

F32 = mybir.dt.float32
BF16 = mybir.dt.bfloat16
AF = mybir.ActivationFunctionType

B, L, H, E = 2, 2048, 16, 64
D = 64
SCALE = 0.125          # 1/sqrt(64)
NCORES = 8
BH_PER_CORE = (B * H) // NCORES   # 4
PACKS = BH_PER_CORE // 2          # 2
NT = L // 128                     # 16 s-tiles
NJ = L // 512                     # 4 l-chunks
VW = 66                           # [V | 1 | 0pad] (even moving width)

_cached = None


def _build_program():
    nc = bacc.Bacc("TRN2", target_bir_lowering=False)
    qt = nc.dram_tensor("qt", [PACKS, 128, L], BF16, kind="ExternalInput")
    kt = nc.dram_tensor("kt", [PACKS, 128, L], BF16, kind="ExternalInput")
    v1d = nc.dram_tensor("v1", [BH_PER_CORE, 128, NT * VW], BF16,
                         kind="ExternalInput")
    ob = nc.dram_tensor("ob", [BH_PER_CORE, NJ, VW, 512], BF16,
                        kind="ExternalOutput")

    with tile.TileContext(nc) as tc:
        with (
            tc.tile_pool(name="consts", bufs=1) as consts,
            tc.tile_pool(name="qk_sb", bufs=2) as qk_sb,
            tc.tile_pool(name="v1_sb", bufs=4) as v1_pool,
            tc.tile_pool(name="pt", bufs=9) as pt_pool,
            tc.tile_pool(name="ot_sb", bufs=2) as ot_sb_pool,
            tc.tile_pool(name="qkps", bufs=5, space="PSUM") as qk_ps,
            tc.tile_pool(name="otps", bufs=3, space="PSUM") as ot_ps,
        ):
            # warm the PE p-state during the input-DMA window with small-N
            # matmuls (ramp needs busy-time, not columns)
            warm_sb = consts.tile([128, 64], BF16)
            nc.vector.memset(warm_sb, 0.25)
            warm_ps = qk_ps.tile([128, 512], F32, tag="pp", name="warm")
            for w in range(6):
                nc.tensor.matmul(
                    warm_ps[0:64, 0:64], warm_sb[:, 0:64], warm_sb,
                    start=True, stop=True, skip_group_check=True,
                )

            pack_tiles = {}

            def load_pack(p):
                # chunk-sliced loads: QK of chunk j only needs q cols
                # [512j, 512j+512) and k cols [0, 512(j+1))
                qt_t = qk_sb.tile([128, NJ, 512], BF16, tag="qt", name="qt_t")
                kt_t = qk_sb.tile([128, NJ, 512], BF16, tag="kt", name="kt_t")
                v1l = []
                for hh2 in range(2):
                    v1l.append(v1_pool.tile([128, NJ, 4 * VW], BF16, tag="v1",
                                            name="v1_t"))
                for j in range(NJ):
                    if j == 0:
                        # piece 0 only needs k cols [0:128); land them first
                        nc.sync.dma_start(out=kt_t[:, 0, 0:128],
                                          in_=kt[p, :, 0:128])
                        nc.sync.dma_start(out=kt_t[:, 0, 128:512],
                                          in_=kt[p, :, 128:512])
                    else:
                        nc.sync.dma_start(out=kt_t[:, j, :],
                                          in_=kt[p, :, 512 * j:512 * (j + 1)])
                    # q loads dispatch from the (initially idle) Act queue so
                    # the first q/k slices transfer in parallel
                    nc.scalar.dma_start(out=qt_t[:, j, :],
                                        in_=qt[p, :, 512 * j:512 * (j + 1)])
                    for hh2 in range(2):
                        eng = nc.gpsimd
                        eng.dma_start(
                            out=v1l[hh2][:, j, :],
                            in_=v1d[2 * p + hh2, :,
                                    4 * VW * j:4 * VW * (j + 1)],
                        )
                qf = qt_t.rearrange("p j c -> p (j c)")
                kf = kt_t.rearrange("p j c -> p (j c)")
                v1f = [v.rearrange("p j c -> p (j c)") for v in v1l]
                pack_tiles[p] = (qf, kf, v1f)

            load_pack(0)
            for pack in range(PACKS):
                qt_t, kt_t, v1 = pack_tiles.pop(pack)

                for j in range(NJ):
                    if j == 2 and pack + 1 < PACKS:
                        load_pack(pack + 1)
                    lo = 512 * j
                    nk = 4 * j + 4          # s_tiles participating causally
                    ot = []
                    for hh in range(2):
                        ot_t = ot_ps.tile([VW, 512], F32, tag="ot", name="ot")
                        ot.append(ot_t)

                    pending_pv = []

                    def emit_pv(hh, k, pt_t, j=j, ot=ot, v1=v1):
                        poff = 128 * max(0, k - 4 * j)
                        nc.tensor.matmul(
                            ot[hh][:, poff:512],
                            v1[hh][:, VW * k:VW * (k + 1)],
                            pt_t[:, poff:512],
                            start=(k == 0),
                            stop=(k == 4 * j + 3),
                            skip_group_check=True,
                        )

                    for k in range(nk):             # s_tile pieces
                        pps, pts = [], []
                        for hh in range(2):
                            pps.append(qk_ps.tile([128, 512], F32, tag="pp",
                                                  name="pp"))
                            pts.append(pt_pool.tile([128, 512], BF16,
                                                    tag="pt", name="pt"))
                        # QK (diag pieces skip the fully-masked leading cols)
                        qoff = 128 * max(0, k - 4 * j)
                        for hh in range(2):
                            r0 = 64 * hh
                            nc.tensor.matmul(
                                pps[hh][:, qoff:512],
                                kt_t[r0:r0 + 64, 128 * k:128 * (k + 1)],
                                qt_t[r0:r0 + 64, lo + qoff:lo + 512],
                                start=True, stop=True,
                            )
                        # delayed PV from pieces ago keeps PE fed; drain
                        # more eagerly on the final chunk to shorten the tail
                        depth = 4 if (pack == PACKS - 1 and j == NJ - 1) else 8
                        while len(pending_pv) > depth:
                            pending_pv.pop(0)()
                        # exp + causal wedge fixups (fixups off PE's path)
                        m = k - 4 * j
                        for hh in range(2):
                            pp, pt_t = pps[hh], pts[hh]
                            if m < 0:                  # plain piece
                                nc.scalar.activation(pt_t, pp, AF.Exp,
                                                     scale=SCALE)
                            else:                      # diagonal piece
                                nc.scalar.activation(
                                    pt_t[:, 128 * m:512], pp[:, 128 * m:512],
                                    AF.Exp, scale=SCALE,
                                )
                                # triangle: keep where f >= p else 1.0
                                nc.gpsimd.affine_select(
                                    out=pt_t[:, 128 * m:128 * m + 128],
                                    in_=pt_t[:, 128 * m:128 * m + 128],
                                    compare_op=mybir.AluOpType.is_ge,
                                    fill=1.0,
                                    base=0,
                                    pattern=[[1, 128]],
                                    channel_multiplier=-1,
                                )
                            pending_pv.append(
                                lambda hh=hh, k=k, p=pts[hh]: emit_pv(hh, k, p)
                            )
                    for fn in pending_pv:
                        fn()
                    pending_pv = []

                    # ship the untransposed [66, 512] accumulator; suffix
                    # add + divide + transpose happen on the host
                    for hh in range(2):
                        bh = 2 * pack + hh
                        ot_s = ot_sb_pool.tile([VW, 512], BF16, tag="ots")
                        nc.vector.tensor_copy(ot_s, ot[hh])
                        nc.sync.dma_start(out=ob[bh, j], in_=ot_s)

    nc.compile()
    return nc


def _get_program():
    global _cached
    if _cached is None:
        _cached = _build_program()
    return _cached


def _shard_inputs(queries, keys, values):
    import ml_dtypes
    BF = ml_dtypes.bfloat16
    # [B, L, H, E] -> [B, H, E, L] -> [BH, E, L]
    qT = np.ascontiguousarray(queries.transpose(0, 2, 3, 1)).reshape(B * H, E, L)
    kT = np.ascontiguousarray(keys.transpose(0, 2, 3, 1)).reshape(B * H, E, L)
    # [B, L, H, D] -> [BH, L, D]
    vv = np.ascontiguousarray(values.transpose(0, 2, 1, 3)).reshape(B * H, L, D)
    in_maps = []
    sufs = []
    for c in range(NCORES):
        s = c * BH_PER_CORE
        qp = qT[s:s + BH_PER_CORE].reshape(PACKS, 128, L)
        kp = kT[s:s + BH_PER_CORE].reshape(PACKS, 128, L)
        vb = vv[s:s + BH_PER_CORE].reshape(BH_PER_CORE, NT, 128, D)
        v1h = np.zeros((BH_PER_CORE, 128, NT, VW), dtype=np.float32)
        v1h[:, :, :, 0:64] = vb.transpose(0, 2, 1, 3)
        v1h[:, :, :, 64] = 1.0
        # suffix tables (f32, exact V), applied host-side at gather:
        # SUF[t] = sum over s_tiles > t of [V|1|0] rows (col 64 = count)
        vrows = v1h.transpose(0, 2, 1, 3).sum(axis=2)  # [BH, NT, VW]
        suf = np.zeros((BH_PER_CORE, NT, VW), dtype=np.float32)
        suf[:, :-1] = vrows[:, ::-1].cumsum(axis=1)[:, -2::-1]
        sufs.append(suf)
        in_maps.append({
            "qt": np.ascontiguousarray(qp).astype(BF),
            "kt": np.ascontiguousarray(kp).astype(BF),
            "v1": v1h.astype(BF).reshape(BH_PER_CORE, 128, NT * VW),
        })
    return in_maps, sufs


def _gather_outputs(results, sufs):
    outs = []
    for r, suf in zip(results, sufs):
        acc = np.ascontiguousarray(r["ob"], dtype=np.float32)             # [BH, NJ, VW, 512]
        acc = acc.transpose(0, 2, 1, 3).reshape(BH_PER_CORE, VW, L)
        sufe = np.repeat(suf.transpose(0, 2, 1), 128, axis=2)  # [BH, VW, L]
        acc += sufe
        o = acc[:, 0:64, :] / acc[:, 64:65, :]       # [BH, D, L]
        outs.append(o.transpose(0, 2, 1))            # [BH, L, D]
    full = np.concatenate(outs, axis=0)              # [B*H, L, D]
    return np.ascontiguousarray(
        full.reshape(B, H, L, D).transpose(0, 2, 1, 3)
    ).astype(np.float32)  # [B, L, H, D]


def kernel(queries, keys, values, _trace=[False]):
    from concourse.bass_utils import run_bass_kernel_spmd

    queries = np.asarray(queries, dtype=np.float32)
    keys = np.asarray(keys, dtype=np.float32)
    values = np.asarray(values, dtype=np.float32)
    nc = _get_program()
    in_maps, sufs = _shard_inputs(queries, keys, values)
    res = run_bass_kernel_spmd(
        nc, in_maps, core_ids=list(range(NCORES)), trace=_trace[0]
    )
    out = _gather_outputs(res.results, sufs)
    if _trace[0]:
        kernel.last_results = res
    return out


# revision 39
# speedup vs baseline: 1.7431x; 1.7431x over previous
"""FullAttention (non-standard multiplicative causal mask) on 8 TRN2 cores.

Reference (per batch b, head h):
    S = Q @ K^T                      [L, L]
    S = S * tril(ones)               (multiplicative mask: zeros above diag)
    A = softmax(S / sqrt(E))         (masked slots contribute exp(0)=1)
    O = A @ V

Key observation: for s > l, P[l,s] = exp(0) = 1, so
    num[l]   = sum_{s<=l} exp(z_ls) v_s  +  sum_{s>l} v_s
    denom[l] = sum_{s<=l} exp(z_ls)      +  (L-1-l)

Sharding: B*H = 32 (b,h) pairs -> 4 per core (2 "packs" of 2 heads).
Host pre-transposes Q,K to [e, l] layout (bf16); 2 heads are stacked on
the 128 SBUF partitions so QK^T matmuls (contraction E=64) row-pack the
PE array.  Q/K/V are loaded in chunk-sized slices so the first QK
matmul starts as soon as the first 512 columns land.

Per (b,h), chunk-outer flash-style loop over l-chunks of 512:
  - QK: S^T pieces [s_tile=128, <=512] via matmul(lhsT=kT, rhs=qT)
  - exp on ScalarE (PSUM->SBUF, bf16 out), causal wedge fixup via
    gpsimd affine_select (exp(0)=1 for s>l inside the diagonal tile)
  - PV: out^T [66, 512] += V1[s_tile].T @ P^T piece  (V1 = [V | 1 | 0]
    adds the softmax denominator as column 64)
  - the [66, 512] accumulator is copied to SBUF bf16 and DMA'd out
    untransposed; the host adds the precomputed suffix sums
    (s_tiles > l's tile), divides by the denominator and transposes.

The PE is power-throttled to ~50% sustained utilization on TRN2, so
total streamed matmul columns (2x causal area) set the span floor; the
warmup ramps the p-state during the input-DMA window using small-N
matmuls (ramp needs busy-time, not columns; columns burn throttle
credit).
"""

import numpy as np

import concourse.bass as bass
import concourse.mybir as mybir
import concourse.tile as tile
from concourse import bacc

F32 = mybir.dt.float32
BF16 = mybir.dt.bfloat16
AF = mybir.ActivationFunctionType

B, L, H, E = 2, 2048, 16, 64
D = 64
SCALE = 0.125          # 1/sqrt(64)
NCORES = 8
BH_PER_CORE = (B * H) // NCORES   # 4
PACKS = BH_PER_CORE // 2          # 2
NT = L // 128                     # 16 s-tiles
NJ = L // 512                     # 4 l-chunks
VW = 66                           # [V | 1 | 0pad] (even moving width)

_cached = None


def _build_program():
    nc = bacc.Bacc("TRN2", target_bir_lowering=False)
    qt = nc.dram_tensor("qt", [PACKS, 128, L], BF16, kind="ExternalInput")
    kt = nc.dram_tensor("kt", [PACKS, 128, L], BF16, kind="ExternalInput")
    v1d = nc.dram_tensor("v1", [BH_PER_CORE, 128, NT * VW], BF16,
                         kind="ExternalInput")
    ob = nc.dram_tensor("ob", [BH_PER_CORE, NJ, VW, 512], BF16,
                        kind="ExternalOutput")

    with tile.TileContext(nc) as tc:
        with (
            tc.tile_pool(name="consts", bufs=1) as consts,
            tc.tile_pool(name="qk_sb", bufs=2) as qk_sb,
            tc.tile_pool(name="v1_sb", bufs=4) as v1_pool,
            tc.tile_pool(name="pt", bufs=10) as pt_pool,
            tc.tile_pool(name="ot_sb", bufs=4) as ot_sb_pool,
            tc.tile_pool(name="qkps", bufs=3, space="PSUM") as qk_ps,
            tc.tile_pool(name="otps", bufs=2, space="PSUM") as ot_ps,
        ):
            # warm the PE p-state during the input-DMA window with small-N
            # matmuls (ramp needs busy-time, not columns)
            warm_sb = consts.tile([128, 64], BF16)
            nc.vector.memset(warm_sb, 0.25)
            warm_ps = qk_ps.tile([128, 2, 512], F32, tag="pp", name="warm")
            for w in range(6):
                nc.tensor.matmul(
                    warm_ps[0:64, 0, 0:64], warm_sb[:, 0:64], warm_sb,
                    start=True, stop=True, skip_group_check=True,
                )

            pack_tiles = {}

            def load_pack(p):
                # chunk-sliced loads: QK of chunk j only needs q cols
                # [512j, 512j+512) and k cols [0, 512(j+1))
                qt_t = qk_sb.tile([128, NJ, 512], BF16, tag="qt", name="qt_t")
                kt_t = qk_sb.tile([128, NJ, 512], BF16, tag="kt", name="kt_t")
                v1l = []
                for hh2 in range(2):
                    v1l.append(v1_pool.tile([128, NJ, 4 * VW], BF16, tag="v1",
                                            name="v1_t"))
                for j in range(NJ):
                    nc.sync.dma_start(out=kt_t[:, j, :],
                                      in_=kt[p, :, 512 * j:512 * (j + 1)])
                    nc.sync.dma_start(out=qt_t[:, j, :],
                                      in_=qt[p, :, 512 * j:512 * (j + 1)])
                    for hh2 in range(2):
                        eng = nc.gpsimd
                        eng.dma_start(
                            out=v1l[hh2][:, j, :],
                            in_=v1d[2 * p + hh2, :,
                                    4 * VW * j:4 * VW * (j + 1)],
                        )
                qf = qt_t.rearrange("p j c -> p (j c)")
                kf = kt_t.rearrange("p j c -> p (j c)")
                v1f = [v.rearrange("p j c -> p (j c)") for v in v1l]
                pack_tiles[p] = (qf, kf, v1f)

            load_pack(0)
            for pack in range(PACKS):
                qt_t, kt_t, v1 = pack_tiles.pop(pack)

                for j in range(NJ):
                    if j == 2 and pack + 1 < PACKS:
                        load_pack(pack + 1)
                    lo = 512 * j
                    nk = 4 * j + 4          # s_tiles participating causally
                    ot = []
                    for hh in range(2):
                        ot_t = ot_ps.tile([VW, 512], F32, tag="ot", name="ot")
                        ot.append(ot_t)

                    pending_pv = []

                    def emit_pv(hh, k, pt_t, j=j, ot=ot, v1=v1):
                        poff = 128 * max(0, k - 4 * j)
                        nc.tensor.matmul(
                            ot[hh][:, poff:512],
                            v1[hh][:, VW * k:VW * (k + 1)],
                            pt_t[:, hh, poff:512],
                            start=(k == 0),
                            stop=(k == 4 * j + 3),
                            skip_group_check=True,
                        )

                    for k in range(nk):             # s_tile pieces
                        # both heads' pieces share one 2-bank PSUM pair and
                        # one SBUF pt pair -> ONE merged exp per piece and
                        # half the pool-rotation semaphore traffic
                        pp = qk_ps.tile([128, 2, 512], F32, tag="pp",
                                        name="pp")
                        pt_t = pt_pool.tile([128, 2, 512], BF16, tag="pt",
                                            name="pt")
                        # QK (diag pieces skip the fully-masked leading cols)
                        qoff = 128 * max(0, k - 4 * j)
                        for hh in range(2):
                            r0 = 64 * hh
                            nc.tensor.matmul(
                                pp[:, hh, qoff:512],
                                kt_t[r0:r0 + 64, 128 * k:128 * (k + 1)],
                                qt_t[r0:r0 + 64, lo + qoff:lo + 512],
                                start=True, stop=True,
                            )
                        # delayed PV from pieces ago keeps PE fed; drain
                        # more eagerly on the final chunk to shorten the tail
                        depth = 4 if (pack == PACKS - 1 and j == NJ - 1) else 8
                        while len(pending_pv) > depth:
                            pending_pv.pop(0)()
                        # merged exp over both heads' halves + causal wedge
                        # fixups (fixups off PE's path)
                        m = k - 4 * j
                        nc.scalar.activation(
                            pt_t[:, :, qoff:512], pp[:, :, qoff:512],
                            AF.Exp, scale=SCALE,
                        )
                        for hh in range(2):
                            if m >= 0:             # diagonal piece
                                # triangle: keep where f >= p else 1.0
                                nc.gpsimd.affine_select(
                                    out=pt_t[:, hh, 128 * m:128 * m + 128],
                                    in_=pt_t[:, hh, 128 * m:128 * m + 128],
                                    compare_op=mybir.AluOpType.is_ge,
                                    fill=1.0,
                                    base=0,
                                    pattern=[[1, 128]],
                                    channel_multiplier=-1,
                                )
                            pending_pv.append(
                                lambda hh=hh, k=k, p=pt_t: emit_pv(hh, k, p)
                            )
                    for fn in pending_pv:
                        fn()
                    pending_pv = []

                    # ship the untransposed [66, 512] accumulator; suffix
                    # add + divide + transpose happen on the host
                    for hh in range(2):
                        bh = 2 * pack + hh
                        ot_s = ot_sb_pool.tile([VW, 512], BF16, tag="ots")
                        nc.vector.tensor_copy(ot_s, ot[hh])
                        nc.sync.dma_start(out=ob[bh, j], in_=ot_s)

    nc.compile()
    return nc


def _get_program():
    global _cached
    if _cached is None:
        _cached = _build_program()
    return _cached


def _shard_inputs(queries, keys, values):
    import ml_dtypes
    BF = ml_dtypes.bfloat16
    # [B, L, H, E] -> [B, H, E, L] -> [BH, E, L]
    qT = np.ascontiguousarray(queries.transpose(0, 2, 3, 1)).reshape(B * H, E, L)
    kT = np.ascontiguousarray(keys.transpose(0, 2, 3, 1)).reshape(B * H, E, L)
    # [B, L, H, D] -> [BH, L, D]
    vv = np.ascontiguousarray(values.transpose(0, 2, 1, 3)).reshape(B * H, L, D)
    in_maps = []
    sufs = []
    for c in range(NCORES):
        s = c * BH_PER_CORE
        qp = qT[s:s + BH_PER_CORE].reshape(PACKS, 128, L)
        kp = kT[s:s + BH_PER_CORE].reshape(PACKS, 128, L)
        vb = vv[s:s + BH_PER_CORE].reshape(BH_PER_CORE, NT, 128, D)
        v1h = np.zeros((BH_PER_CORE, 128, NT, VW), dtype=np.float32)
        v1h[:, :, :, 0:64] = vb.transpose(0, 2, 1, 3)
        v1h[:, :, :, 64] = 1.0
        # suffix tables (f32, exact V), applied host-side at gather:
        # SUF[t] = sum over s_tiles > t of [V|1|0] rows (col 64 = count)
        vrows = v1h.transpose(0, 2, 1, 3).sum(axis=2)  # [BH, NT, VW]
        suf = np.zeros((BH_PER_CORE, NT, VW), dtype=np.float32)
        suf[:, :-1] = vrows[:, ::-1].cumsum(axis=1)[:, -2::-1]
        sufs.append(suf)
        in_maps.append({
            "qt": np.ascontiguousarray(qp).astype(BF),
            "kt": np.ascontiguousarray(kp).astype(BF),
            "v1": v1h.astype(BF).reshape(BH_PER_CORE, 128, NT * VW),
        })
    return in_maps, sufs


def _gather_outputs(results, sufs):
    outs = []
    for r, suf in zip(results, sufs):
        acc = np.ascontiguousarray(r["ob"], dtype=np.float32)             # [BH, NJ, VW, 512]
        acc = acc.transpose(0, 2, 1, 3).reshape(BH_PER_CORE, VW, L)
        sufe = np.repeat(suf.transpose(0, 2, 1), 128, axis=2)  # [BH, VW, L]
        acc += sufe
        o = acc[:, 0:64, :] / acc[:, 64:65, :]       # [BH, D, L]
        outs.append(o.transpose(0, 2, 1))            # [BH, L, D]
    full = np.concatenate(outs, axis=0)              # [B*H, L, D]
    return np.ascontiguousarray(
        full.reshape(B, H, L, D).transpose(0, 2, 1, 3)
    ).astype(np.float32)  # [B, L, H, D]


def kernel(queries, keys, values, _trace=[False]):
    from concourse.bass_utils import run_bass_kernel_spmd

    queries = np.asarray(queries, dtype=np.float32)
    keys = np.asarray(keys, dtype=np.float32)
    values = np.asarray(values, dtype=np.float32)
    nc = _get_program()
    in_maps, sufs = _shard_inputs(queries, keys, values)
    res = run_bass_kernel_spmd(
        nc, in_maps, core_ids=list(range(NCORES)), trace=_trace[0]
    )
    out = _gather_outputs(res.results, sufs)
    if _trace[0]:
        kernel.last_results = res
    return out


# revision 40
# speedup vs baseline: 1.7833x; 1.0231x over previous
"""FullAttention (non-standard multiplicative causal mask) on 8 TRN2 cores.

Reference (per batch b, head h):
    S = Q @ K^T                      [L, L]
    S = S * tril(ones)               (multiplicative mask: zeros above diag)
    A = softmax(S / sqrt(E))         (masked slots contribute exp(0)=1)
    O = A @ V

Key observation: for s > l, P[l,s] = exp(0) = 1, so
    num[l]   = sum_{s<=l} exp(z_ls) v_s  +  sum_{s>l} v_s
    denom[l] = sum_{s<=l} exp(z_ls)      +  (L-1-l)

Sharding: B*H = 32 (b,h) pairs -> 4 per core (2 "packs" of 2 heads).
Host pre-transposes Q,K to [e, l] layout (bf16); 2 heads are stacked on
the 128 SBUF partitions so QK^T matmuls (contraction E=64) row-pack the
PE array.  Q/K/V are loaded in chunk-sized slices so the first QK
matmul starts as soon as the first 512 columns land.

Per (b,h), chunk-outer flash-style loop over l-chunks of 512:
  - QK: S^T pieces [s_tile=128, <=512] via matmul(lhsT=kT, rhs=qT)
  - exp on ScalarE (PSUM->SBUF, bf16 out), causal wedge fixup via
    gpsimd affine_select (exp(0)=1 for s>l inside the diagonal tile)
  - PV: out^T [66, 512] += V1[s_tile].T @ P^T piece  (V1 = [V | 1 | 0]
    adds the softmax denominator as column 64)
  - the [66, 512] accumulator is copied to SBUF bf16 and DMA'd out
    untransposed; the host adds the precomputed suffix sums
    (s_tiles > l's tile), divides by the denominator and transposes.

The PE is power-throttled to ~50% sustained utilization on TRN2, so
total streamed matmul columns (2x causal area) set the span floor; the
warmup ramps the p-state during the input-DMA window using small-N
matmuls (ramp needs busy-time, not columns; columns burn throttle
credit).
"""

import numpy as np

import concourse.bass as bass
import concourse.mybir as mybir
import concourse.tile as tile
from concourse import bacc

F32 = mybir.dt.float32
BF16 = mybir.dt.bfloat16
AF = mybir.ActivationFunctionType

B, L, H, E = 2, 2048, 16, 64
D = 64
SCALE = 0.125          # 1/sqrt(64)
NCORES = 8
BH_PER_CORE = (B * H) // NCORES   # 4
PACKS = BH_PER_CORE // 2          # 2
NT = L // 128                     # 16 s-tiles
NJ = L // 512                     # 4 l-chunks
VW = 66                           # [V | 1 | 0pad] (even moving width)

_cached = None


def _build_program():
    nc = bacc.Bacc("TRN2", target_bir_lowering=False)
    qt = nc.dram_tensor("qt", [PACKS, 128, L], BF16, kind="ExternalInput")
    kt = nc.dram_tensor("kt", [PACKS, 128, L], BF16, kind="ExternalInput")
    v1d = nc.dram_tensor("v1", [BH_PER_CORE, 128, NT * VW], BF16,
                         kind="ExternalInput")
    ob = nc.dram_tensor("ob", [BH_PER_CORE, NJ, VW, 512], BF16,
                        kind="ExternalOutput")

    with tile.TileContext(nc) as tc:
        with (
            tc.tile_pool(name="consts", bufs=1) as consts,
            tc.tile_pool(name="qk_sb", bufs=2) as qk_sb,
            tc.tile_pool(name="v1_sb", bufs=4) as v1_pool,
            tc.tile_pool(name="pt", bufs=10) as pt_pool,
            tc.tile_pool(name="ot_sb", bufs=4) as ot_sb_pool,
            tc.tile_pool(name="qkps", bufs=3, space="PSUM") as qk_ps,
            tc.tile_pool(name="otps", bufs=2, space="PSUM") as ot_ps,
        ):
            # warm the PE p-state during the input-DMA window with small-N
            # matmuls (ramp needs busy-time, not columns)
            warm_sb = consts.tile([128, 64], BF16)
            nc.vector.memset(warm_sb, 0.25)
            warm_ps = qk_ps.tile([128, 2, 512], F32, tag="pp", name="warm")
            for w in range(6):
                nc.tensor.matmul(
                    warm_ps[0:64, 0, 0:64], warm_sb[:, 0:64], warm_sb,
                    start=True, stop=True, skip_group_check=True,
                )

            pack_tiles = {}

            def load_pack(p):
                # chunk-sliced loads: QK of chunk j only needs q cols
                # [512j, 512j+512) and k cols [0, 512(j+1))
                qt_t = qk_sb.tile([128, NJ, 512], BF16, tag="qt", name="qt_t")
                kt_t = qk_sb.tile([128, NJ, 512], BF16, tag="kt", name="kt_t")
                v1l = []
                for hh2 in range(2):
                    v1l.append(v1_pool.tile([128, NJ, 4 * VW], BF16, tag="v1",
                                            name="v1_t"))
                for j in range(NJ):
                    # k on sync, q on gpsimd: the first slices of both land
                    # in parallel so the first QK starts ~2us earlier
                    nc.sync.dma_start(out=kt_t[:, j, :],
                                      in_=kt[p, :, 512 * j:512 * (j + 1)])
                    nc.gpsimd.dma_start(out=qt_t[:, j, :],
                                        in_=qt[p, :, 512 * j:512 * (j + 1)])
                    for hh2 in range(2):
                        nc.sync.dma_start(
                            out=v1l[hh2][:, j, :],
                            in_=v1d[2 * p + hh2, :,
                                    4 * VW * j:4 * VW * (j + 1)],
                        )
                qf = qt_t.rearrange("p j c -> p (j c)")
                kf = kt_t.rearrange("p j c -> p (j c)")
                v1f = [v.rearrange("p j c -> p (j c)") for v in v1l]
                pack_tiles[p] = (qf, kf, v1f)

            load_pack(0)
            for pack in range(PACKS):
                qt_t, kt_t, v1 = pack_tiles.pop(pack)

                for j in range(NJ):
                    if j == 2 and pack + 1 < PACKS:
                        load_pack(pack + 1)
                    lo = 512 * j
                    nk = 4 * j + 4          # s_tiles participating causally
                    ot = []
                    for hh in range(2):
                        ot_t = ot_ps.tile([VW, 512], F32, tag="ot", name="ot")
                        ot.append(ot_t)

                    pending_pv = []

                    def emit_pv(hh, k, pt_t, j=j, ot=ot, v1=v1):
                        poff = 128 * max(0, k - 4 * j)
                        nc.tensor.matmul(
                            ot[hh][:, poff:512],
                            v1[hh][:, VW * k:VW * (k + 1)],
                            pt_t[:, hh, poff:512],
                            start=(k == 0),
                            stop=(k == 4 * j + 3),
                            skip_group_check=True,
                        )

                    for k in range(nk):             # s_tile pieces
                        # both heads' pieces share one 2-bank PSUM pair and
                        # one SBUF pt pair -> ONE merged exp per piece and
                        # half the pool-rotation semaphore traffic
                        pp = qk_ps.tile([128, 2, 512], F32, tag="pp",
                                        name="pp")
                        pt_t = pt_pool.tile([128, 2, 512], BF16, tag="pt",
                                            name="pt")
                        # QK (diag pieces skip the fully-masked leading cols)
                        qoff = 128 * max(0, k - 4 * j)
                        for hh in range(2):
                            r0 = 64 * hh
                            nc.tensor.matmul(
                                pp[:, hh, qoff:512],
                                kt_t[r0:r0 + 64, 128 * k:128 * (k + 1)],
                                qt_t[r0:r0 + 64, lo + qoff:lo + 512],
                                start=True, stop=True,
                            )
                        # delayed PV from pieces ago keeps PE fed; drain
                        # more eagerly on the final chunk to shorten the tail
                        depth = 4 if (pack == PACKS - 1 and j == NJ - 1) else 8
                        while len(pending_pv) > depth:
                            pending_pv.pop(0)()
                        # merged exp over both heads' halves + causal wedge
                        # fixups (fixups off PE's path)
                        m = k - 4 * j
                        nc.scalar.activation(
                            pt_t[:, :, qoff:512], pp[:, :, qoff:512],
                            AF.Exp, scale=SCALE,
                        )
                        for hh in range(2):
                            if m >= 0:             # diagonal piece
                                # triangle: keep where f >= p else 1.0
                                nc.gpsimd.affine_select(
                                    out=pt_t[:, hh, 128 * m:128 * m + 128],
                                    in_=pt_t[:, hh, 128 * m:128 * m + 128],
                                    compare_op=mybir.AluOpType.is_ge,
                                    fill=1.0,
                                    base=0,
                                    pattern=[[1, 128]],
                                    channel_multiplier=-1,
                                )
                            pending_pv.append(
                                lambda hh=hh, k=k, p=pt_t: emit_pv(hh, k, p)
                            )
                    for fn in pending_pv:
                        fn()
                    pending_pv = []

                    # ship the untransposed [66, 512] accumulator; suffix
                    # add + divide + transpose happen on the host
                    for hh in range(2):
                        bh = 2 * pack + hh
                        ot_s = ot_sb_pool.tile([VW, 512], BF16, tag="ots")
                        nc.vector.tensor_copy(ot_s, ot[hh])
                        nc.sync.dma_start(out=ob[bh, j], in_=ot_s)

    nc.compile()
    return nc


def _get_program():
    global _cached
    if _cached is None:
        _cached = _build_program()
    return _cached


def _shard_inputs(queries, keys, values):
    import ml_dtypes
    BF = ml_dtypes.bfloat16
    # [B, L, H, E] -> [B, H, E, L] -> [BH, E, L]
    qT = np.ascontiguousarray(queries.transpose(0, 2, 3, 1)).reshape(B * H, E, L)
    kT = np.ascontiguousarray(keys.transpose(0, 2, 3, 1)).reshape(B * H, E, L)
    # [B, L, H, D] -> [BH, L, D]
    vv = np.ascontiguousarray(values.transpose(0, 2, 1, 3)).reshape(B * H, L, D)
    in_maps = []
    sufs = []
    for c in range(NCORES):
        s = c * BH_PER_CORE
        qp = qT[s:s + BH_PER_CORE].reshape(PACKS, 128, L)
        kp = kT[s:s + BH_PER_CORE].reshape(PACKS, 128, L)
        vb = vv[s:s + BH_PER_CORE].reshape(BH_PER_CORE, NT, 128, D)
        v1h = np.zeros((BH_PER_CORE, 128, NT, VW), dtype=np.float32)
        v1h[:, :, :, 0:64] = vb.transpose(0, 2, 1, 3)
        v1h[:, :, :, 64] = 1.0
        # suffix tables (f32, exact V), applied host-side at gather:
        # SUF[t] = sum over s_tiles > t of [V|1|0] rows (col 64 = count)
        vrows = v1h.transpose(0, 2, 1, 3).sum(axis=2)  # [BH, NT, VW]
        suf = np.zeros((BH_PER_CORE, NT, VW), dtype=np.float32)
        suf[:, :-1] = vrows[:, ::-1].cumsum(axis=1)[:, -2::-1]
        sufs.append(suf)
        in_maps.append({
            "qt": np.ascontiguousarray(qp).astype(BF),
            "kt": np.ascontiguousarray(kp).astype(BF),
            "v1": v1h.astype(BF).reshape(BH_PER_CORE, 128, NT * VW),
        })
    return in_maps, sufs


def _gather_outputs(results, sufs):
    outs = []
    for r, suf in zip(results, sufs):
        acc = np.ascontiguousarray(r["ob"], dtype=np.float32)             # [BH, NJ, VW, 512]
        acc = acc.transpose(0, 2, 1, 3).reshape(BH_PER_CORE, VW, L)
        sufe = np.repeat(suf.transpose(0, 2, 1), 128, axis=2)  # [BH, VW, L]
        acc += sufe
        o = acc[:, 0:64, :] / acc[:, 64:65, :]       # [BH, D, L]
        outs.append(o.transpose(0, 2, 1))            # [BH, L, D]
    full = np.concatenate(outs, axis=0)              # [B*H, L, D]
    return np.ascontiguousarray(
        full.reshape(B, H, L, D).transpose(0, 2, 1, 3)
    ).astype(np.float32)  # [B, L, H, D]


def kernel(queries, keys, values, _trace=[False]):
    from concourse.bass_utils import run_bass_kernel_spmd

    queries = np.asarray(queries, dtype=np.float32)
    keys = np.asarray(keys, dtype=np.float32)
    values = np.asarray(values, dtype=np.float32)
    nc = _get_program()
    in_maps, sufs = _shard_inputs(queries, keys, values)
    res = run_bass_kernel_spmd(
        nc, in_maps, core_ids=list(range(NCORES)), trace=_trace[0]
    )
    out = _gather_outputs(res.results, sufs)
    if _trace[0]:
        kernel.last_results = res
    return out


# revision 42
# speedup vs baseline: 1.7877x; 1.0025x over previous
"""FullAttention (non-standard multiplicative causal mask) on 8 TRN2 cores.

Reference (per batch b, head h):
    S = Q @ K^T                      [L, L]
    S = S * tril(ones)               (multiplicative mask: zeros above diag)
    A = softmax(S / sqrt(E))         (masked slots contribute exp(0)=1)
    O = A @ V

Key observation: for s > l, P[l,s] = exp(0) = 1, so
    num[l]   = sum_{s<=l} exp(z_ls) v_s  +  sum_{s>l} v_s
    denom[l] = sum_{s<=l} exp(z_ls)      +  (L-1-l)

Sharding: B*H = 32 (b,h) pairs -> 4 per core (2 "packs" of 2 heads).
Host pre-transposes Q,K to [e, l] layout (bf16); 2 heads are stacked on
the 128 SBUF partitions so QK^T matmuls (contraction E=64) row-pack the
PE array.  Q/K/V are loaded in chunk-sized slices so the first QK
matmul starts as soon as the first 512 columns land.

Per (b,h), chunk-outer flash-style loop over l-chunks of 512:
  - QK: S^T pieces [s_tile=128, <=512] via matmul(lhsT=kT, rhs=qT)
  - exp on ScalarE (PSUM->SBUF, bf16 out), causal wedge fixup via
    gpsimd affine_select (exp(0)=1 for s>l inside the diagonal tile)
  - PV: out^T [66, 512] += V1[s_tile].T @ P^T piece  (V1 = [V | 1 | 0]
    adds the softmax denominator as column 64)
  - the [66, 512] accumulator is copied to SBUF bf16 and DMA'd out
    untransposed; the host adds the precomputed suffix sums
    (s_tiles > l's tile), divides by the denominator and transposes.

The PE is power-throttled to ~50% sustained utilization on TRN2, so
total streamed matmul columns (2x causal area) set the span floor; the
warmup ramps the p-state during the input-DMA window using small-N
matmuls (ramp needs busy-time, not columns; columns burn throttle
credit).
"""

import numpy as np

import concourse.bass as bass
import concourse.mybir as mybir
import concourse.tile as tile
from concourse import bacc

F32 = mybir.dt.float32
BF16 = mybir.dt.bfloat16
AF = mybir.ActivationFunctionType

B, L, H, E = 2, 2048, 16, 64
D = 64
SCALE = 0.125          # 1/sqrt(64)
NCORES = 8
BH_PER_CORE = (B * H) // NCORES   # 4
PACKS = BH_PER_CORE // 2          # 2
NT = L // 128                     # 16 s-tiles
NJ = L // 512                     # 4 l-chunks
VW = 66                           # [V | 1 | 0pad] (even moving width)

_cached = None


def _build_program():
    nc = bacc.Bacc("TRN2", target_bir_lowering=False)
    qt = nc.dram_tensor("qt", [PACKS, 128, L], BF16, kind="ExternalInput")
    kt = nc.dram_tensor("kt", [PACKS, 128, L], BF16, kind="ExternalInput")
    v1d = nc.dram_tensor("v1", [BH_PER_CORE, 128, NT * VW], BF16,
                         kind="ExternalInput")
    ob = nc.dram_tensor("ob", [BH_PER_CORE, NJ, VW, 512], BF16,
                        kind="ExternalOutput")

    with tile.TileContext(nc) as tc:
        with (
            tc.tile_pool(name="consts", bufs=1) as consts,
            tc.tile_pool(name="qk_sb", bufs=2) as qk_sb,
            tc.tile_pool(name="v1_sb", bufs=4) as v1_pool,
            tc.tile_pool(name="pt", bufs=10) as pt_pool,
            tc.tile_pool(name="ot_sb", bufs=4) as ot_sb_pool,
            tc.tile_pool(name="qkps", bufs=3, space="PSUM") as qk_ps,
            tc.tile_pool(name="otps", bufs=2, space="PSUM") as ot_ps,
        ):
            # warm the PE p-state during the input-DMA window with small-N
            # matmuls (ramp needs busy-time, not columns)
            warm_sb = consts.tile([128, 64], BF16)
            nc.vector.memset(warm_sb, 0.25)
            warm_ps = qk_ps.tile([128, 2, 512], F32, tag="pp", name="warm")
            for w in range(6):
                nc.tensor.matmul(
                    warm_ps[0:64, 0, 0:64], warm_sb[:, 0:64], warm_sb,
                    start=True, stop=True, skip_group_check=True,
                )

            pack_tiles = {}

            def load_pack(p):
                # chunk-sliced loads: QK of chunk j only needs q cols
                # [512j, 512j+512) and k cols [0, 512(j+1))
                qt_t = qk_sb.tile([128, NJ, 512], BF16, tag="qt", name="qt_t")
                kt_t = qk_sb.tile([128, NJ, 512], BF16, tag="kt", name="kt_t")
                v1l = []
                for hh2 in range(2):
                    v1l.append(v1_pool.tile([128, NJ, 4 * VW], BF16, tag="v1",
                                            name="v1_t"))
                for j in range(NJ):
                    # k on sync, q on gpsimd: the first slices of both land
                    # in parallel so the first QK starts ~2us earlier;
                    # piece 0 only needs k cols [0:128), so land those first
                    if j == 0:
                        nc.sync.dma_start(out=kt_t[:, 0, 0:128],
                                          in_=kt[p, :, 0:128])
                        nc.sync.dma_start(out=kt_t[:, 0, 128:512],
                                          in_=kt[p, :, 128:512])
                    else:
                        nc.sync.dma_start(out=kt_t[:, j, :],
                                          in_=kt[p, :, 512 * j:512 * (j + 1)])
                    nc.gpsimd.dma_start(out=qt_t[:, j, :],
                                        in_=qt[p, :, 512 * j:512 * (j + 1)])
                    for hh2 in range(2):
                        nc.sync.dma_start(
                            out=v1l[hh2][:, j, :],
                            in_=v1d[2 * p + hh2, :,
                                    4 * VW * j:4 * VW * (j + 1)],
                        )
                qf = qt_t.rearrange("p j c -> p (j c)")
                kf = kt_t.rearrange("p j c -> p (j c)")
                v1f = [v.rearrange("p j c -> p (j c)") for v in v1l]
                pack_tiles[p] = (qf, kf, v1f)

            load_pack(0)
            for pack in range(PACKS):
                qt_t, kt_t, v1 = pack_tiles.pop(pack)

                for j in range(NJ):
                    if j == 2 and pack + 1 < PACKS:
                        load_pack(pack + 1)
                    lo = 512 * j
                    nk = 4 * j + 4          # s_tiles participating causally
                    ot = []
                    for hh in range(2):
                        ot_t = ot_ps.tile([VW, 512], F32, tag="ot", name="ot")
                        ot.append(ot_t)

                    pending_pv = []

                    def emit_pv(hh, k, pt_t, j=j, ot=ot, v1=v1):
                        poff = 128 * max(0, k - 4 * j)
                        nc.tensor.matmul(
                            ot[hh][:, poff:512],
                            v1[hh][:, VW * k:VW * (k + 1)],
                            pt_t[:, hh, poff:512],
                            start=(k == 0),
                            stop=(k == 4 * j + 3),
                            skip_group_check=True,
                        )

                    for k in range(nk):             # s_tile pieces
                        # both heads' pieces share one 2-bank PSUM pair and
                        # one SBUF pt pair -> ONE merged exp per piece and
                        # half the pool-rotation semaphore traffic
                        pp = qk_ps.tile([128, 2, 512], F32, tag="pp",
                                        name="pp")
                        pt_t = pt_pool.tile([128, 2, 512], BF16, tag="pt",
                                            name="pt")
                        # QK (diag pieces skip the fully-masked leading cols)
                        qoff = 128 * max(0, k - 4 * j)
                        for hh in range(2):
                            r0 = 64 * hh
                            nc.tensor.matmul(
                                pp[:, hh, qoff:512],
                                kt_t[r0:r0 + 64, 128 * k:128 * (k + 1)],
                                qt_t[r0:r0 + 64, lo + qoff:lo + 512],
                                start=True, stop=True,
                            )
                        # delayed PV from pieces ago keeps PE fed; drain
                        # more eagerly on the final chunk to shorten the tail
                        depth = 2 if (pack == PACKS - 1 and j == NJ - 1) else 8
                        while len(pending_pv) > depth:
                            pending_pv.pop(0)()
                        # merged exp over both heads' halves + causal wedge
                        # fixups (fixups off PE's path)
                        m = k - 4 * j
                        nc.scalar.activation(
                            pt_t[:, :, qoff:512], pp[:, :, qoff:512],
                            AF.Exp, scale=SCALE,
                        )
                        for hh in range(2):
                            if m >= 0:             # diagonal piece
                                # triangle: keep where f >= p else 1.0
                                nc.gpsimd.affine_select(
                                    out=pt_t[:, hh, 128 * m:128 * m + 128],
                                    in_=pt_t[:, hh, 128 * m:128 * m + 128],
                                    compare_op=mybir.AluOpType.is_ge,
                                    fill=1.0,
                                    base=0,
                                    pattern=[[1, 128]],
                                    channel_multiplier=-1,
                                )
                            pending_pv.append(
                                lambda hh=hh, k=k, p=pt_t: emit_pv(hh, k, p)
                            )
                    for fn in pending_pv:
                        fn()
                    pending_pv = []

                    # ship the untransposed [66, 512] accumulator; suffix
                    # add + divide + transpose happen on the host
                    for hh in range(2):
                        bh = 2 * pack + hh
                        ot_s = ot_sb_pool.tile([VW, 512], BF16, tag="ots")
                        nc.vector.tensor_copy(ot_s, ot[hh])
                        nc.sync.dma_start(out=ob[bh, j], in_=ot_s)

    nc.compile()
    return nc


def _get_program():
    global _cached
    if _cached is None:
        _cached = _build_program()
    return _cached


def _shard_inputs(queries, keys, values):
    import ml_dtypes
    BF = ml_dtypes.bfloat16
    # [B, L, H, E] -> [B, H, E, L] -> [BH, E, L]
    qT = np.ascontiguousarray(queries.transpose(0, 2, 3, 1)).reshape(B * H, E, L)
    kT = np.ascontiguousarray(keys.transpose(0, 2, 3, 1)).reshape(B * H, E, L)
    # [B, L, H, D] -> [BH, L, D]
    vv = np.ascontiguousarray(values.transpose(0, 2, 1, 3)).reshape(B * H, L, D)
    in_maps = []
    sufs = []
    for c in range(NCORES):
        s = c * BH_PER_CORE
        qp = qT[s:s + BH_PER_CORE].reshape(PACKS, 128, L)
        kp = kT[s:s + BH_PER_CORE].reshape(PACKS, 128, L)
        vb = vv[s:s + BH_PER_CORE].reshape(BH_PER_CORE, NT, 128, D)
        v1h = np.zeros((BH_PER_CORE, 128, NT, VW), dtype=np.float32)
        v1h[:, :, :, 0:64] = vb.transpose(0, 2, 1, 3)
        v1h[:, :, :, 64] = 1.0
        # suffix tables (f32, exact V), applied host-side at gather:
        # SUF[t] = sum over s_tiles > t of [V|1|0] rows (col 64 = count)
        vrows = v1h.transpose(0, 2, 1, 3).sum(axis=2)  # [BH, NT, VW]
        suf = np.zeros((BH_PER_CORE, NT, VW), dtype=np.float32)
        suf[:, :-1] = vrows[:, ::-1].cumsum(axis=1)[:, -2::-1]
        sufs.append(suf)
        in_maps.append({
            "qt": np.ascontiguousarray(qp).astype(BF),
            "kt": np.ascontiguousarray(kp).astype(BF),
            "v1": v1h.astype(BF).reshape(BH_PER_CORE, 128, NT * VW),
        })
    return in_maps, sufs


def _gather_outputs(results, sufs):
    outs = []
    for r, suf in zip(results, sufs):
        acc = np.ascontiguousarray(r["ob"], dtype=np.float32)             # [BH, NJ, VW, 512]
        acc = acc.transpose(0, 2, 1, 3).reshape(BH_PER_CORE, VW, L)
        sufe = np.repeat(suf.transpose(0, 2, 1), 128, axis=2)  # [BH, VW, L]
        acc += sufe
        o = acc[:, 0:64, :] / acc[:, 64:65, :]       # [BH, D, L]
        outs.append(o.transpose(0, 2, 1))            # [BH, L, D]
    full = np.concatenate(outs, axis=0)              # [B*H, L, D]
    return np.ascontiguousarray(
        full.reshape(B, H, L, D).transpose(0, 2, 1, 3)
    ).astype(np.float32)  # [B, L, H, D]


def kernel(queries, keys, values, _trace=[False]):
    from concourse.bass_utils import run_bass_kernel_spmd

    queries = np.asarray(queries, dtype=np.float32)
    keys = np.asarray(keys, dtype=np.float32)
    values = np.asarray(values, dtype=np.float32)
    nc = _get_program()
    in_maps, sufs = _shard_inputs(queries, keys, values)
    res = run_bass_kernel_spmd(
        nc, in_maps, core_ids=list(range(NCORES)), trace=_trace[0]
    )
    out = _gather_outputs(res.results, sufs)
    if _trace[0]:
        kernel.last_results = res
    return out


# revision 44
# speedup vs baseline: 1.8251x; 1.0209x over previous
"""FullAttention (non-standard multiplicative causal mask) on 8 TRN2 cores.

Reference (per batch b, head h):
    S = Q @ K^T                      [L, L]
    S = S * tril(ones)               (multiplicative mask: zeros above diag)
    A = softmax(S / sqrt(E))         (masked slots contribute exp(0)=1)
    O = A @ V

Key observation: for s > l, P[l,s] = exp(0) = 1, so
    num[l]   = sum_{s<=l} exp(z_ls) v_s  +  sum_{s>l} v_s
    denom[l] = sum_{s<=l} exp(z_ls)      +  (L-1-l)

Sharding: B*H = 32 (b,h) pairs -> 4 per core (2 "packs" of 2 heads).
Host pre-transposes Q,K to [e, l] layout (bf16); 2 heads are stacked on
the 128 SBUF partitions so QK^T matmuls (contraction E=64) row-pack the
PE array.  Q/K/V are loaded in chunk-sized slices so the first QK
matmul starts as soon as the first 512 columns land.

Per (b,h), chunk-outer flash-style loop over l-chunks of 512:
  - QK: S^T pieces [s_tile=128, <=512] via matmul(lhsT=kT, rhs=qT)
  - exp on ScalarE (PSUM->SBUF, bf16 out), causal wedge fixup via
    gpsimd affine_select (exp(0)=1 for s>l inside the diagonal tile)
  - PV: out^T [66, 512] += V1[s_tile].T @ P^T piece  (V1 = [V | 1 | 0]
    adds the softmax denominator as column 64)
  - the [66, 512] accumulator is copied to SBUF bf16 and DMA'd out
    untransposed; the host adds the precomputed suffix sums
    (s_tiles > l's tile), divides by the denominator and transposes.

The PE is power-throttled to ~50% sustained utilization on TRN2, so
total streamed matmul columns (2x causal area) set the span floor; the
warmup ramps the p-state during the input-DMA window using small-N
matmuls (ramp needs busy-time, not columns; columns burn throttle
credit).
"""

import numpy as np

import concourse.bass as bass
import concourse.mybir as mybir
import concourse.tile as tile
from concourse import bacc

F32 = mybir.dt.float32
BF16 = mybir.dt.bfloat16
AF = mybir.ActivationFunctionType

B, L, H, E = 2, 2048, 16, 64
D = 64
SCALE = 0.125          # 1/sqrt(64)
NCORES = 8
BH_PER_CORE = (B * H) // NCORES   # 4
PACKS = BH_PER_CORE // 2          # 2
NT = L // 128                     # 16 s-tiles
NJ = L // 512                     # 4 l-chunks
VW = 66                           # [V | 1 | 0pad] (even moving width)

_cached = None


def _build_program():
    nc = bacc.Bacc("TRN2", target_bir_lowering=False)
    qt = nc.dram_tensor("qt", [PACKS, 128, L], BF16, kind="ExternalInput")
    kt = nc.dram_tensor("kt", [PACKS, 128, L], BF16, kind="ExternalInput")
    v1d = nc.dram_tensor("v1", [BH_PER_CORE, 128, NT * VW], BF16,
                         kind="ExternalInput")
    ob = nc.dram_tensor("ob", [BH_PER_CORE, NJ, VW, 512], BF16,
                        kind="ExternalOutput")

    with tile.TileContext(nc) as tc:
        with (
            tc.tile_pool(name="consts", bufs=1) as consts,
            tc.tile_pool(name="qk_sb", bufs=2) as qk_sb,
            tc.tile_pool(name="v1_sb", bufs=4) as v1_pool,
            tc.tile_pool(name="pt", bufs=10) as pt_pool,
            tc.tile_pool(name="ot_sb", bufs=4) as ot_sb_pool,
            tc.tile_pool(name="qkps", bufs=3, space="PSUM") as qk_ps,
            tc.tile_pool(name="otps", bufs=2, space="PSUM") as ot_ps,
        ):
            # warm the PE p-state during the input-DMA window with small-N
            # matmuls (ramp needs busy-time, not columns)
            warm_sb = consts.tile([128, 64], BF16)
            nc.vector.memset(warm_sb, 0.25)
            warm_ps = qk_ps.tile([128, 2, 512], F32, tag="pp", name="warm")
            for w in range(6):
                nc.tensor.matmul(
                    warm_ps[0:64, 0, 0:64], warm_sb[:, 0:64], warm_sb,
                    start=True, stop=True, skip_group_check=True,
                )

            pack_tiles = {}

            def load_pack(p):
                # chunk-sliced loads: QK of chunk j only needs q cols
                # [512j, 512j+512) and k cols [0, 512(j+1))
                qt_t = qk_sb.tile([128, NJ, 512], BF16, tag="qt", name="qt_t")
                kt_t = qk_sb.tile([128, NJ, 512], BF16, tag="kt", name="kt_t")
                v1l = []
                for hh2 in range(2):
                    v1l.append(v1_pool.tile([128, NJ, 4 * VW], BF16, tag="v1",
                                            name="v1_t"))
                for j in range(NJ):
                    # k on sync, q on gpsimd: the first slices of both land
                    # in parallel so the first QK starts ~2us earlier;
                    # piece 0 only needs k cols [0:128), so land those first
                    if j == 0:
                        nc.sync.dma_start(out=kt_t[:, 0, 0:128],
                                          in_=kt[p, :, 0:128])
                        nc.sync.dma_start(out=kt_t[:, 0, 128:512],
                                          in_=kt[p, :, 128:512])
                    else:
                        nc.sync.dma_start(out=kt_t[:, j, :],
                                          in_=kt[p, :, 512 * j:512 * (j + 1)])
                    nc.gpsimd.dma_start(out=qt_t[:, j, :],
                                        in_=qt[p, :, 512 * j:512 * (j + 1)])
                    for hh2 in range(2):
                        nc.sync.dma_start(
                            out=v1l[hh2][:, j, :],
                            in_=v1d[2 * p + hh2, :,
                                    4 * VW * j:4 * VW * (j + 1)],
                        )
                qf = qt_t.rearrange("p j c -> p (j c)")
                kf = kt_t.rearrange("p j c -> p (j c)")
                v1f = [v.rearrange("p j c -> p (j c)") for v in v1l]
                pack_tiles[p] = (qf, kf, v1f)

            load_pack(0)
            for pack in range(PACKS):
                qt_t, kt_t, v1 = pack_tiles.pop(pack)

                # pack 0 ascending (matches incremental load arrival); pack 1
                # descending: its inputs are fully prefetched, the deepest
                # chunk pipelines best right after the pack transition, and
                # ending on the 4-piece chunk shortens the drain tail
                j_iter = range(NJ) if pack == 0 else range(NJ - 1, -1, -1)
                for j in j_iter:
                    if pack == 0 and j == 2 and pack + 1 < PACKS:
                        load_pack(pack + 1)
                    lo = 512 * j
                    nk = 4 * j + 4          # s_tiles participating causally
                    ot = []
                    for hh in range(2):
                        ot_t = ot_ps.tile([VW, 512], F32, tag="ot", name="ot")
                        ot.append(ot_t)

                    pending_pv = []

                    def emit_pv(hh, k, pt_t, j=j, ot=ot, v1=v1):
                        poff = 128 * max(0, k - 4 * j)
                        nc.tensor.matmul(
                            ot[hh][:, poff:512],
                            v1[hh][:, VW * k:VW * (k + 1)],
                            pt_t[:, hh, poff:512],
                            start=(k == 0),
                            stop=(k == 4 * j + 3),
                            skip_group_check=True,
                        )

                    for k in range(nk):             # s_tile pieces
                        # both heads' pieces share one 2-bank PSUM pair and
                        # one SBUF pt pair -> ONE merged exp per piece and
                        # half the pool-rotation semaphore traffic
                        pp = qk_ps.tile([128, 2, 512], F32, tag="pp",
                                        name="pp")
                        pt_t = pt_pool.tile([128, 2, 512], BF16, tag="pt",
                                            name="pt")
                        # QK (diag pieces skip the fully-masked leading cols)
                        qoff = 128 * max(0, k - 4 * j)
                        for hh in range(2):
                            r0 = 64 * hh
                            nc.tensor.matmul(
                                pp[:, hh, qoff:512],
                                kt_t[r0:r0 + 64, 128 * k:128 * (k + 1)],
                                qt_t[r0:r0 + 64, lo + qoff:lo + 512],
                                start=True, stop=True,
                            )
                        # delayed PV from pieces ago keeps PE fed; drain
                        # more eagerly on the final chunk to shorten the tail
                        depth = 2 if (pack == PACKS - 1 and j == 0) else 8
                        while len(pending_pv) > depth:
                            pending_pv.pop(0)()
                        # merged exp over both heads' halves + causal wedge
                        # fixups (fixups off PE's path)
                        m = k - 4 * j
                        nc.scalar.activation(
                            pt_t[:, :, qoff:512], pp[:, :, qoff:512],
                            AF.Exp, scale=SCALE,
                        )
                        for hh in range(2):
                            if m >= 0:             # diagonal piece
                                # triangle: keep where f >= p else 1.0
                                nc.gpsimd.affine_select(
                                    out=pt_t[:, hh, 128 * m:128 * m + 128],
                                    in_=pt_t[:, hh, 128 * m:128 * m + 128],
                                    compare_op=mybir.AluOpType.is_ge,
                                    fill=1.0,
                                    base=0,
                                    pattern=[[1, 128]],
                                    channel_multiplier=-1,
                                )
                            pending_pv.append(
                                lambda hh=hh, k=k, p=pt_t: emit_pv(hh, k, p)
                            )
                    for fn in pending_pv:
                        fn()
                    pending_pv = []

                    # ship the untransposed [66, 512] accumulator; suffix
                    # add + divide + transpose happen on the host
                    for hh in range(2):
                        bh = 2 * pack + hh
                        ot_s = ot_sb_pool.tile([VW, 512], BF16, tag="ots")
                        nc.vector.tensor_copy(ot_s, ot[hh])
                        nc.sync.dma_start(out=ob[bh, j], in_=ot_s)

    nc.compile()
    return nc


def _get_program():
    global _cached
    if _cached is None:
        _cached = _build_program()
    return _cached


def _shard_inputs(queries, keys, values):
    import ml_dtypes
    BF = ml_dtypes.bfloat16
    # [B, L, H, E] -> [B, H, E, L] -> [BH, E, L]
    qT = np.ascontiguousarray(queries.transpose(0, 2, 3, 1)).reshape(B * H, E, L)
    kT = np.ascontiguousarray(keys.transpose(0, 2, 3, 1)).reshape(B * H, E, L)
    # [B, L, H, D] -> [BH, L, D]
    vv = np.ascontiguousarray(values.transpose(0, 2, 1, 3)).reshape(B * H, L, D)
    in_maps = []
    sufs = []
    for c in range(NCORES):
        s = c * BH_PER_CORE
        qp = qT[s:s + BH_PER_CORE].reshape(PACKS, 128, L)
        kp = kT[s:s + BH_PER_CORE].reshape(PACKS, 128, L)
        vb = vv[s:s + BH_PER_CORE].reshape(BH_PER_CORE, NT, 128, D)
        v1h = np.zeros((BH_PER_CORE, 128, NT, VW), dtype=np.float32)
        v1h[:, :, :, 0:64] = vb.transpose(0, 2, 1, 3)
        v1h[:, :, :, 64] = 1.0
        # suffix tables (f32, exact V), applied host-side at gather:
        # SUF[t] = sum over s_tiles > t of [V|1|0] rows (col 64 = count)
        vrows = v1h.transpose(0, 2, 1, 3).sum(axis=2)  # [BH, NT, VW]
        suf = np.zeros((BH_PER_CORE, NT, VW), dtype=np.float32)
        suf[:, :-1] = vrows[:, ::-1].cumsum(axis=1)[:, -2::-1]
        sufs.append(suf)
        in_maps.append({
            "qt": np.ascontiguousarray(qp).astype(BF),
            "kt": np.ascontiguousarray(kp).astype(BF),
            "v1": v1h.astype(BF).reshape(BH_PER_CORE, 128, NT * VW),
        })
    return in_maps, sufs


def _gather_outputs(results, sufs):
    outs = []
    for r, suf in zip(results, sufs):
        acc = np.ascontiguousarray(r["ob"], dtype=np.float32)             # [BH, NJ, VW, 512]
        acc = acc.transpose(0, 2, 1, 3).reshape(BH_PER_CORE, VW, L)
        sufe = np.repeat(suf.transpose(0, 2, 1), 128, axis=2)  # [BH, VW, L]
        acc += sufe
        o = acc[:, 0:64, :] / acc[:, 64:65, :]       # [BH, D, L]
        outs.append(o.transpose(0, 2, 1))            # [BH, L, D]
    full = np.concatenate(outs, axis=0)              # [B*H, L, D]
    return np.ascontiguousarray(
        full.reshape(B, H, L, D).transpose(0, 2, 1, 3)
    ).astype(np.float32)  # [B, L, H, D]


def kernel(queries, keys, values, _trace=[False]):
    from concourse.bass_utils import run_bass_kernel_spmd

    queries = np.asarray(queries, dtype=np.float32)
    keys = np.asarray(keys, dtype=np.float32)
    values = np.asarray(values, dtype=np.float32)
    nc = _get_program()
    in_maps, sufs = _shard_inputs(queries, keys, values)
    res = run_bass_kernel_spmd(
        nc, in_maps, core_ids=list(range(NCORES)), trace=_trace[0]
    )
    out = _gather_outputs(res.results, sufs)
    if _trace[0]:
        kernel.last_results = res
    return out
